# revision 1
# baseline (speedup 1.0000x reference)
"""Trainium2 Bass kernel for nn_CHPS_model_20976620273883 (retrieval_knn).

Computes, for x[8192,4096] f32, W[4096,1024] f32, b[1024] f32,
prototypes[1000,1024] f32:

    emb   = x @ W + b
    cos   = normalize(emb) @ normalize(prototypes).T
    out   = (cos - 1) / 0.01            # == 100*cos - 100

Sharding: data-parallel on the batch — each of the 8 NeuronCores gets
1024 rows of x; W / b / prototypes are replicated.  No collectives.

Device algorithm (per core), all matmuls in bf16 with fp32 PSUM accum:
  phase 1: embT[D,Bl] = W.T @ x.T    (W k-slices stationary, xT moving;
           xT produced by 2-byte xbar DMA-transpose straight from DRAM)
  norms:   q[b] = sum_d (embT[d,b]+bias)^2 via ACT Square + DVE adds,
           PE-transpose of the partial sums + DVE row-reduce, then
           s100[b] = 1/sqrt(q*1e-4) = 100/||emb_b||  (ACT Sqrt + DVE recip)
  phase 2: raw[Bl,P] = embT.T @ protoT_n  (embT slices stationary,
           prototypes normalized on-chip, transposed via 2-byte xbar DMA)
  epilogue: out = raw*s100[b] - 100     (one DVE tensor_scalar from PSUM)
"""

import numpy as np
import ml_dtypes

B, F_IN, D, P = 8192, 4096, 1024, 1000
NCORES = 8
BL = B // NCORES          # 1024 rows per core
KT = F_IN // 128          # 32 contraction tiles
DT = D // 128             # 8 embedding-dim tiles
NB = 512                  # phase-1 moving width (one fp32 PSUM bank)
NCH = BL // NB            # 2 batch chunks per core
PT = 128                  # proto rows per natural tile
P_PAD = 1024              # prototypes padded to 8 tiles of 128

_cache = {}


def _emit(nc, tc, mybir, x_d, w_d, b_d, p_d, o_d, id_f32):
    f32 = mybir.dt.float32
    bf16 = mybir.dt.bfloat16
    AF = mybir.ActivationFunctionType
    Alu = mybir.AluOpType

    with (
        tc.tile_pool(name="const", bufs=1) as constp,
        tc.tile_pool(name="wpool", bufs=1) as wpool,
        tc.tile_pool(name="xpool", bufs=1) as xpool,
        tc.tile_pool(name="embp", bufs=1) as embp,
        tc.tile_pool(name="ptp", bufs=1) as ptp,
        tc.tile_pool(name="pnat", bufs=2) as pnat,
        tc.tile_pool(name="work", bufs=3) as work,
        tc.tile_pool(name="sml", bufs=2) as sml,
        tc.tile_pool(name="outp", bufs=4) as outp,
        tc.tile_pool(name="ps1", bufs=4, space="PSUM") as ps1p,
        tc.tile_pool(name="ps2", bufs=2, space="PSUM") as ps2p,
        tc.tile_pool(name="pst", bufs=2, space="PSUM") as pstp,
    ):
        # ---- constants -------------------------------------------------
        idf = constp.tile([128, 128], f32)
        nc.sync.dma_start(idf[:], id_f32.ap())
        # b rearranged to column layout: bcol[p, d] = b[d*128 + p]
        bcol = constp.tile([128, DT], f32)
        nc.sync.dma_start(bcol[:], b_d.ap().rearrange("(d p) -> p d", p=128))
        # tiny epsilon bias tile (emulates reference's max(norm, eps) clamp
        # and keeps the zero-padded prototype rows NaN-free)
        epsb = constp.tile([128, 1], f32)
        nc.vector.memset(epsb[:], 1e-24)

        # ---- weight / xT loads. Tile serializes xbar-mode transitions
        # (copy DMA <-> transpose DMA, ~19us drain each in the cost model),
        # so keep one clean transition: all W copies, then all transposes.
        wt = []
        for k in range(KT):
            wtk = wpool.tile([128, D], bf16, name=f"w{k}")
            nc.sync.dma_start(wtk[:], w_d.ap()[k * 128:(k + 1) * 128, :])
            wt.append(wtk)
        xt = []
        for k in range(KT):
            xtk = xpool.tile([128, BL], bf16, name=f"xT{k}")
            nc.sync.dma_start(
                xtk[:], x_d.ap()[:, k * 128:(k + 1) * 128], transpose=True
            )
            xt.append(xtk)

        # persistent bf16 embT tiles: embT[t] rows = emb dims t*128..t*128+127
        embt = [embp.tile([128, BL], bf16, name=f"embT{t}") for t in range(DT)]
        # transposed normalized prototypes: ptt[t][:, p] = proto_n[p, t*128+..]
        ptt = [ptp.tile([128, P_PAD], bf16, name=f"ptT{t}") for t in range(DT)]

        # per-chunk 100/||emb_b|| columns  (s100[cc][:, j] for b-tile cc*4+j)
        s100 = [sml.tile([128, 4], f32, name=f"s100_{cc}") for cc in range(NCH)]

        # ================= phase 1: embT = W.T @ xT =====================
        def phase1_chunk(cc):
            bs = cc * NB
            partial = work.tile([128, NB], f32, name=f"psum_sq{cc}", tag="partial")
            for d in range(DT):
                ps = ps1p.tile([128, NB], f32, name="ps1")
                for k in range(KT):
                    nc.tensor.matmul(
                        ps[:],
                        wt[k][:, d * 128:(d + 1) * 128],
                        xt[k][:, bs:bs + NB],
                        start=(k == 0),
                        stop=(k == KT - 1),
                    )
                # emb (bias added) -> bf16 for phase 2
                nc.vector.tensor_scalar(
                    embt[d][:, bs:bs + NB], ps[:], bcol[:, d:d + 1], None, Alu.add
                )
                # squared emb (bias folded into ACT) -> f32
                sq = work.tile([128, NB], f32, name="sq", tag="sq")
                nc.scalar.activation(
                    sq[:], ps[:], AF.Square, bias=bcol[:, d:d + 1], scale=1.0
                )
                if d == 0:
                    nc.vector.tensor_copy(partial[:], sq[:])
                else:
                    nc.vector.tensor_add(partial[:], partial[:], sq[:])
            # norms: transpose partial 128x128 blocks, reduce rows
            qcol = sml.tile([128, 4], f32, name=f"qcol{cc}", tag="qcol")
            for j in range(4):
                pt = pstp.tile([128, 128], f32, name="pst", tag="tp")
                nc.tensor.transpose(pt[:], partial[:, j * 128:(j + 1) * 128], idf[:])
                nc.vector.tensor_reduce(
                    qcol[:, j:j + 1], pt[:], mybir.AxisListType.X, Alu.add
                )
            # s100 = 1/sqrt(q*1e-4 + eps) = 100/||emb||   (clamp-safe)
            rt = sml.tile([128, 4], f32, name=f"rt{cc}", tag="rt")
            nc.scalar.activation(rt[:], qcol[:], AF.Sqrt, bias=epsb[:], scale=1e-4)
            nc.vector.reciprocal(s100[cc][:], rt[:])

        # ================= phase 0b: prototypes =========================
        # All copy-DMAs + normalization first, then every transpose-DMA in
        # one block: xbar-mode transitions (copy<->transpose) serialize the
        # DMA stream, so keep them to a minimum.
        def proto_prep():
            pnns = []
            for t in range(DT):
                pn = pnat.tile([128, D], bf16, name=f"pn{t}", tag="pn")
                rows = min(PT, P - t * PT)
                if rows < PT:
                    nc.vector.memset(pn[:], 0.0)
                nc.sync.dma_start(
                    pn[:rows, :], p_d.ap()[t * PT:t * PT + rows, :]
                )
                psq = work.tile([128, D], f32, name="psq", tag="psq", bufs=1)
                nc.scalar.activation(psq[:], pn[:], AF.Square)
                pq = sml.tile([128, 1], f32, name="pq", tag="pq")
                nc.vector.tensor_reduce(
                    pq[:], psq[:], mybir.AxisListType.X, Alu.add
                )
                pr = sml.tile([128, 1], f32, name="pr", tag="pq")
                nc.scalar.activation(pr[:], pq[:], AF.Sqrt, bias=epsb[:])
                pri = sml.tile([128, 1], f32, name="pri", tag="pq")
                nc.vector.reciprocal(pri[:], pr[:])
                pnn = pnat.tile([128, D], bf16, name=f"pnn{t}", tag="pnn", bufs=DT)
                nc.vector.tensor_scalar(pnn[:], pn[:], pri[:], None, Alu.mult)
                pnns.append(pnn)
            for t in range(DT):
                for c in range(DT):
                    # 2-byte xbar SBUF->SBUF transpose keeps this off the PE
                    nc.sync.dma_start(
                        ptt[c][:, t * 128:(t + 1) * 128],
                        pnns[t][:, c * 128:(c + 1) * 128],
                        transpose=True,
                    )

        # ================= phase 2: out = embT.T @ protoT ===============
        def phase2_chunk(cc):
            for j in range(4):
                bt = cc * 4 + j
                for pc, (pn0, pnn_) in enumerate([(0, NB), (NB, P - NB)]):
                    ps2 = ps2p.tile([128, NB], f32, name="ps2")
                    for t in range(DT):
                        nc.tensor.matmul(
                            ps2[:, :pnn_],
                            embt[t][:, bt * 128:(bt + 1) * 128],
                            ptt[t][:, pn0:pn0 + pnn_],
                            start=(t == 0),
                            stop=(t == DT - 1),
                        )
                    ot = outp.tile([128, NB], f32, name="ot")
                    nc.vector.tensor_scalar(
                        ot[:, :pnn_], ps2[:, :pnn_], s100[cc][:, j:j + 1],
                        -100.0, Alu.mult, Alu.add,
                    )
                    nc.sync.dma_start(
                        o_d.ap()[bt * 128:(bt + 1) * 128, pn0:pn0 + pnn_],
                        ot[:, :pnn_],
                    )

        # emission order: big chunk-0 matmul first so the proto pipeline
        # (DMA/ACT/DVE) and its PE transposes hide under it.
        phase1_chunk(0)
        proto_prep()
        phase2_chunk(0)
        phase1_chunk(1)
        phase2_chunk(1)


def _build(reps=1):
    key = ("mod", reps)
    if key in _cache:
        return _cache[key]
    import concourse.bacc as bacc
    import concourse.mybir as mybir
    import concourse.tile as tile

    nc = bacc.Bacc(
        "TRN2", target_bir_lowering=False, debug=False, num_devices=NCORES
    )
    f32 = mybir.dt.float32
    bf16 = mybir.dt.bfloat16
    x_d = nc.dram_tensor("x", [BL, F_IN], bf16, kind="ExternalInput")
    w_d = nc.dram_tensor("w", [F_IN, D], bf16, kind="ExternalInput")
    b_d = nc.dram_tensor("b", [D], f32, kind="ExternalInput")
    p_d = nc.dram_tensor("protos", [P, D], bf16, kind="ExternalInput")
    o_d = nc.dram_tensor("out", [BL, P], f32, kind="ExternalOutput")
    id_f32 = nc.inline_tensor(np.eye(128, dtype=np.float32), name="id_f32")

    with tile.TileContext(nc) as tc:
        for _ in range(reps):
            _emit(nc, tc, mybir, x_d, w_d, b_d, p_d, o_d, id_f32)
    nc.compile()
    _cache[key] = nc
    return nc


def _in_maps(inputs):
    x = np.ascontiguousarray(inputs["x"]).astype(ml_dtypes.bfloat16)
    w = np.ascontiguousarray(inputs["W"]).astype(ml_dtypes.bfloat16)
    bb = np.ascontiguousarray(inputs["b"]).astype(np.float32)
    pp = np.ascontiguousarray(inputs["prototypes"]).astype(ml_dtypes.bfloat16)
    return [
        {"x": x[c * BL:(c + 1) * BL], "w": w, "b": bb, "protos": pp}
        for c in range(NCORES)
    ]


def kernel(**inputs) -> np.ndarray:
    from concourse import bass_utils

    nc = _build(reps=1)
    in_maps = _in_maps(inputs)
    try:
        res = bass_utils.run_bass_kernel_spmd(
            nc, in_maps, core_ids=list(range(NCORES))
        )
    except Exception:
        # transient axon-session hiccups are recoverable on a second attempt
        res = bass_utils.run_bass_kernel_spmd(
            nc, in_maps, core_ids=list(range(NCORES))
        )
    return np.concatenate([res.results[c]["out"] for c in range(NCORES)], axis=0)



# revision 3
# speedup vs baseline: 10.0325x; 10.0325x over previous
"""Trainium2 Bass kernel for nn_CHPS_model_20976620273883 (retrieval_knn).

Computes, for x[8192,4096] f32, W[4096,1024] f32, b[1024] f32,
prototypes[1000,1024] f32:

    emb   = x @ W + b
    cos   = normalize(emb) @ normalize(prototypes).T
    out   = (cos - 1) / 0.01            # == 100*cos - 100

Sharding: data-parallel on the batch — each of the 8 NeuronCores gets
1024 rows of x; W / b / prototypes are replicated.  No collectives.

Device algorithm (per core).  Both matmuls run in fp8(e4m3) DoubleRow
mode (2 contraction tiles per instruction, ~2x bf16 PE throughput);
tolerance is 2e-2 and the fp8 error lands ~2.5e-3, so precision is
traded for the compute roofline.  Scales keep everything in e4m3's
normal range (host sends W*64 so its std is 1.0, not 2^-6):

  phase 1: embT64[D,Bl] = (64*W).T @ x.T   fp8 DoubleRow, f32 PSUM
           (xT pre-transposed on host; all DMAs are plain copies)
           embT fp8 tile = (PSUM + 64*b) / 8          (std ~8)
           q[b] = sum_d (PSUM+64b)^2 via ACT Square + DVE adds,
           PE-transpose + row-reduce, s_b = 1/sqrt(q*0.0064)
                                         = 100/(512*||emb||)
  protos:  q_p = ones128.T @ Square(protoT)  (reduce over partitions
           AND broadcast to all 128 rows in one matmul)
           sb[p] = 1/sqrt(q_p/4096) = 64/||p||; pn8 = protoT*sb (fp8)
  phase 2: raw[Bl,P] = embT.T @ pn8   fp8 DoubleRow
           out = raw*s_b - 100                  (one DVE tensor_scalar)
"""

import numpy as np
import ml_dtypes

B, F_IN, D, P = 8192, 4096, 1024, 1000
NCORES = 8
BL = B // NCORES          # 1024 rows per core
KT = F_IN // 128          # 32 contraction tiles
KP = KT // 2              # 16 k-pairs (DoubleRow)
DT = D // 128             # 8 embedding-dim tiles
TPAIR = DT // 2           # 4 t-pairs (DoubleRow, phase 2)
NB = 512                  # batch-chunk width (one fp32 PSUM bank)
NCH = BL // NB            # 2 batch chunks per core
P_PAD = 1024              # prototypes padded to 8 tiles of 128

SC_W = 64.0               # host-side W scale (keeps fp8 W out of subnormals)
SC_E = 0.125              # PSUM -> fp8 embT scale (E = 8*emb, std ~8)
SC_P = 64.0               # proto direction scale (Q = 64*p_hat, std ~2)
QS_EMB = (SC_E * SC_P / 100.0) ** 2   # 0.0064:  s_b = 1/sqrt(q*QS_EMB)
QS_P = 1.0 / (SC_P * SC_P)            # 1/4096:  sb  = 1/sqrt(q_p*QS_P)

_cache = {}


def _emit(nc, tc, mybir, x_d, wq_d, ptn_d, b64_d, o_d, id_f32):
    f32 = mybir.dt.float32
    bf16 = mybir.dt.bfloat16
    f8 = mybir.dt.float8e4
    AF = mybir.ActivationFunctionType
    Alu = mybir.AluOpType
    DR = mybir.MatmulPerfMode.DoubleRow

    with (
        tc.tile_pool(name="const", bufs=1) as constp,
        tc.tile_pool(name="wpool", bufs=1) as wpool,
        tc.tile_pool(name="xpool", bufs=1) as xpool,
        tc.tile_pool(name="ppool", bufs=1) as ppool,
        tc.tile_pool(name="embp", bufs=1) as embp,
        tc.tile_pool(name="work", bufs=2) as work,
        tc.tile_pool(name="sml", bufs=2) as sml,
        tc.tile_pool(name="outp", bufs=4) as outp,
        tc.tile_pool(name="ps1", bufs=3, space="PSUM") as ps1p,
        tc.tile_pool(name="ps2", bufs=2, space="PSUM") as ps2p,
        tc.tile_pool(name="qps", bufs=2, space="PSUM") as qpsp,
        tc.tile_pool(name="pst", bufs=1, space="PSUM") as pstp,
    ):
        # ---- constants -------------------------------------------------
        idf = constp.tile([128, 128], f32)
        nc.sync.dma_start(idf[:], id_f32.ap())
        onesb = constp.tile([128, 128], bf16)
        nc.vector.memset(onesb[:], 1.0)
        # bcol[p, d] = 64*b[d*128 + p]
        bcol = constp.tile([128, DT], f32)
        nc.sync.dma_start(bcol[:], b64_d.ap().rearrange("(d p) -> p d", p=128))
        # tiny epsilon (keeps zero-padded proto columns NaN-free; emulates
        # the reference's max(norm, eps) clamp)
        epsb = constp.tile([128, 1], f32)
        nc.vector.memset(epsb[:], 1e-24)

        # ---- input loads (all plain copy DMAs, no xbar transposes) -----
        wq = []
        for q in range(4):
            t = wpool.tile([128, KT, 256], f8, name=f"wq{q}")
            nc.sync.dma_start(t[:], wq_d[q].ap())
            wq.append(t)
        xt = []
        for cc in range(NCH):
            t = xpool.tile([128, KT, NB], f8, name=f"x{cc}")
            nc.sync.dma_start(t[:], x_d[cc].ap())
            xt.append(t)
        ptn = ppool.tile([128, DT, P_PAD], bf16, name="ptn")
        nc.sync.dma_start(ptn[:], ptn_d.ap())

        # persistent tiles
        embt = embp.tile([128, DT, BL], f8, name="embt")
        pn8 = ppool.tile([128, DT, P_PAD], f8, name="pn8")
        sb = ppool.tile([128, P_PAD], f32, name="sb")
        s100 = [sml.tile([128, 4], f32, name=f"s100_{cc}") for cc in range(NCH)]

        # ================= phase 1: embT = (64W).T @ xT  (fp8 DR) =======
        def phase1_chunk(cc):
            bs = cc * NB
            partial = work.tile([128, NB], f32, name=f"partial{cc}", tag="partial")
            for d in range(DT):
                dq, dj = divmod(d, 2)
                ps = ps1p.tile([128, NB], f32, name="ps1")
                for kp in range(KP):
                    nc.tensor.matmul(
                        ps[:],
                        wq[dq][:, 2 * kp:2 * kp + 2, dj * 128:(dj + 1) * 128],
                        xt[cc][:, 2 * kp:2 * kp + 2, :],
                        start=(kp == 0),
                        stop=(kp == KP - 1),
                        perf_mode=DR,
                    )
                # embT fp8 = (psum + 64b)/8
                nc.vector.tensor_scalar(
                    embt[:, d, bs:bs + NB], ps[:], bcol[:, d:d + 1], SC_E,
                    Alu.add, Alu.mult,
                )
                # squared scaled emb (bias folded into ACT) -> f32
                sq = work.tile([128, NB], f32, name="sq", tag="sq", bufs=3)
                nc.scalar.activation(
                    sq[:], ps[:], AF.Square, bias=bcol[:, d:d + 1], scale=1.0
                )
                if d == 0:
                    nc.vector.tensor_copy(partial[:], sq[:])
                else:
                    nc.vector.tensor_add(partial[:], partial[:], sq[:])
            return partial

        def norms_chunk(cc, partial):
            # q columns: transpose partial 128x128 blocks, reduce rows
            qcol = sml.tile([128, 4], f32, name=f"qcol{cc}", tag="qcol")
            for j in range(4):
                pt = pstp.tile([128, 128], f32, name="pst", tag="tp")
                nc.tensor.transpose(pt[:], partial[:, j * 128:(j + 1) * 128], idf[:])
                nc.vector.tensor_reduce(
                    qcol[:, j:j + 1], pt[:], mybir.AxisListType.X, Alu.add
                )
            # s_b = 1/sqrt(q*QS_EMB + eps) = 100/(512*||emb||)
            rt = sml.tile([128, 4], f32, name=f"rt{cc}", tag="rt")
            nc.scalar.activation(rt[:], qcol[:], AF.Sqrt, bias=epsb[:], scale=QS_EMB)
            nc.vector.reciprocal(s100[cc][:], rt[:])

        # ================= proto norms + fp8 prescale ===================
        def proto_prep():
            psq = ppool.tile([128, DT, P_PAD], bf16, name="psq")
            nc.scalar.activation(psq[:], ptn[:], AF.Square)
            for pc in range(2):
                qp = qpsp.tile([128, 512], f32, name="qp")
                for t in range(DT):
                    # ones.T @ sq: reduces over partitions AND broadcasts
                    # q_p to all 128 output rows in one matmul
                    nc.tensor.matmul(
                        qp[:], onesb[:], psq[:, t, pc * 512:(pc + 1) * 512],
                        start=(t == 0), stop=(t == DT - 1),
                    )
                srt = work.tile([128, 512], f32, name="srt", tag="srt")
                nc.scalar.activation(srt[:], qp[:], AF.Sqrt, bias=epsb[:], scale=QS_P)
                nc.vector.reciprocal(sb[:, pc * 512:(pc + 1) * 512], srt[:])
            for t in range(DT):
                nc.vector.tensor_tensor(
                    pn8[:, t, :], ptn[:, t, :], sb[:], Alu.mult
                )

        # ================= phase 2: out = embT.T @ pn8  (fp8 DR) ========
        def phase2_chunk(cc):
            for j in range(4):
                bt = cc * 4 + j
                for p0, pw in ((0, 512), (512, P - 512)):
                    ps2 = ps2p.tile([128, NB], f32, name="ps2")
                    for tp in range(TPAIR):
                        nc.tensor.matmul(
                            ps2[:, :pw],
                            embt[:, 2 * tp:2 * tp + 2, bt * 128:(bt + 1) * 128],
                            pn8[:, 2 * tp:2 * tp + 2, p0:p0 + pw],
                            start=(tp == 0),
                            stop=(tp == TPAIR - 1),
                            perf_mode=DR,
                        )
                    ot = outp.tile([128, NB], f32, name="ot")
                    nc.vector.tensor_scalar(
                        ot[:, :pw], ps2[:, :pw], s100[cc][:, j:j + 1],
                        -100.0, Alu.mult, Alu.add,
                    )
                    nc.sync.dma_start(
                        o_d.ap()[bt * 128:(bt + 1) * 128, p0:p0 + pw],
                        ot[:, :pw],
                    )

        # emission order: big chunk-0 matmul first; the proto pipeline
        # (DMA/ACT/DVE + its 16 small PE matmuls) hides under it.
        p0 = phase1_chunk(0)
        proto_prep()
        norms_chunk(0, p0)
        phase2_chunk(0)
        p1 = phase1_chunk(1)
        norms_chunk(1, p1)
        phase2_chunk(1)


def _build(reps=1):
    key = ("mod", reps)
    if key in _cache:
        return _cache[key]
    import concourse.bacc as bacc
    import concourse.mybir as mybir
    import concourse.tile as tile

    nc = bacc.Bacc(
        "TRN2", target_bir_lowering=False, debug=False, num_devices=NCORES
    )
    f32 = mybir.dt.float32
    bf16 = mybir.dt.bfloat16
    f8 = mybir.dt.float8e4
    x_d = [
        nc.dram_tensor(f"x{cc}", [128, KT * NB], f8, kind="ExternalInput")
        for cc in range(NCH)
    ]
    wq_d = [
        nc.dram_tensor(f"wq{q}", [128, KT * 256], f8, kind="ExternalInput")
        for q in range(4)
    ]
    ptn_d = nc.dram_tensor("ptn", [128, DT * P_PAD], bf16, kind="ExternalInput")
    b64_d = nc.dram_tensor("b64", [D], f32, kind="ExternalInput")
    o_d = nc.dram_tensor("out", [BL, P], f32, kind="ExternalOutput")
    id_f32 = nc.inline_tensor(np.eye(128, dtype=np.float32), name="id_f32")

    with tile.TileContext(nc) as tc:
        for _ in range(reps):
            _emit(nc, tc, mybir, x_d, wq_d, ptn_d, b64_d, o_d, id_f32)
    nc.compile()
    _cache[key] = nc
    return nc


def _in_maps(inputs):
    f8 = ml_dtypes.float8_e4m3
    bf16 = ml_dtypes.bfloat16
    x8 = np.ascontiguousarray(inputs["x"]).astype(f8)              # [B, F_IN]
    w64 = (np.ascontiguousarray(inputs["W"]) * SC_W).astype(f8)    # [F_IN, D]
    b64 = (np.ascontiguousarray(inputs["b"]) * SC_W).astype(np.float32)
    pr = np.ascontiguousarray(inputs["prototypes"]).astype(bf16)   # [P, D]

    prp = np.zeros((P_PAD, D), bf16)
    prp[:P] = pr
    # ptn[p, t*P_PAD + pp] = protoT[t*128+p, pp]
    ptn = np.ascontiguousarray(
        prp.T.reshape(DT, 128, P_PAD).transpose(1, 0, 2)
    ).reshape(128, -1)
    # wq[q][p, k*256 + dc] = 64*W[k*128+p, q*256+dc]
    w3 = w64.reshape(KT, 128, D)
    wqs = [
        np.ascontiguousarray(
            w3[:, :, q * 256:(q + 1) * 256].transpose(1, 0, 2)
        ).reshape(128, -1)
        for q in range(4)
    ]
    maps = []
    for c in range(NCORES):
        x3 = np.ascontiguousarray(x8[c * BL:(c + 1) * BL].T).reshape(KT, 128, BL)
        m = {
            "ptn": ptn,
            "b64": b64,
        }
        for q in range(4):
            m[f"wq{q}"] = wqs[q]
        for cc in range(NCH):
            # x{cc}[p, k*NB + n] = x[c*BL + cc*NB + n, k*128 + p]
            m[f"x{cc}"] = np.ascontiguousarray(
                x3[:, :, cc * NB:(cc + 1) * NB].transpose(1, 0, 2)
            ).reshape(128, -1)
        maps.append(m)
    return maps


def kernel(**inputs) -> np.ndarray:
    from concourse import bass_utils

    nc = _build(reps=1)
    in_maps = _in_maps(inputs)
    try:
        res = bass_utils.run_bass_kernel_spmd(
            nc, in_maps, core_ids=list(range(NCORES))
        )
    except Exception:
        # transient axon-session hiccups are recoverable on a second attempt
        res = bass_utils.run_bass_kernel_spmd(
            nc, in_maps, core_ids=list(range(NCORES))
        )
    return np.concatenate([res.results[c]["out"] for c in range(NCORES)], axis=0)
